# revision 20
# baseline (speedup 1.0000x reference)
"""GCNConv (N=50000, E=600000, C=128) on 8 TRN2 NeuronCores via Bass/Tile.

out = scatter_add(norm[e] * x[col[e]] -> row[e]) @ W.T + b,
norm[e] = deg^-1/2[row[e]] * deg^-1/2[col[e]]  (deg over row indices).

Strategy: shard by destination-node range (6250 nodes/core). Host sorts
edges by (core, dest-window, col-bucket) and pads to 128-edge tiles; the
per-(window,bucket) tile budgets are maxed over cores so a single SPMD
program (uniform instruction stream, per-core data) serves all 8 cores.

Per core: dma_gather fetches x rows (fp32, 512B descriptors); the DVE
builds a norm-valued one-hot [128 edges x 64 slots] per tile via
(iota == slot) * norm; the PE accumulates aggT[ch, dest] = msgs^T @
one-hot into PSUM banks (start/stop per bank; per-element has_written
gives overwrite-then-accumulate); banks are copied into an SBUF
aggT [128, 6250]; final PE matmuls apply W^T, a DVE add applies the
(partition-broadcast) bias, and rows DMA out. Output slices concatenate
to the full [50000, 128] result.
"""
import sys
import hashlib

if "/opt/trn_rl_repo" not in sys.path:
    sys.path.insert(0, "/opt/trn_rl_repo")

import numpy as np

P = 128
C = 128
N_NODES = 50000
N_EDGES = 600000
N_CORES = 8


class _Cfg:
    def __init__(self, n_nodes, n_cores=8, S=64, bank_w=8, run_cap=16,
                 lo_lim=32768):
        assert n_nodes % n_cores == 0
        self.N = n_nodes
        self.n_cores = n_cores
        self.NPC = n_nodes // n_cores
        self.S = S
        self.BANK_W = bank_w
        self.GROUP_W = 2 * bank_w
        self.WPC = -(-self.NPC // S)
        self.NGROUPS = -(-self.WPC // self.GROUP_W)
        self.RUN_CAP = run_cap
        self.LO_LIM = min(lo_lim, n_nodes)
        self.HI_BASE = max(0, n_nodes - 32768)
        assert self.LO_LIM <= 32768
        assert self.N - self.HI_BASE <= 32768
        assert self.LO_LIM > self.HI_BASE or self.N <= self.LO_LIM


def _build_plan(cfg, edge_index):
    row = np.asarray(edge_index[0], dtype=np.int64)
    col = np.asarray(edge_index[1], dtype=np.int64)
    N, NPC, S, WPC = cfg.N, cfg.NPC, cfg.S, cfg.WPC
    nco = cfg.n_cores

    deg = np.bincount(row, minlength=N).astype(np.float64)
    dis = np.where(deg > 0, 1.0 / np.sqrt(np.maximum(deg, 1.0)), 0.0)
    norm = (dis[row] * dis[col]).astype(np.float32)

    core = row // NPC
    r_loc = row - core * NPC
    win = r_loc // S
    slot = r_loc % S
    # category: 0 = lo-only (< HI_BASE), 1 = flexible, 2 = hi-only (>= LO_LIM)
    cat = np.where(col < cfg.HI_BASE, 0, np.where(col < cfg.LO_LIM, 1, 2))

    key3 = (core * WPC + win) * 3 + cat
    cnt3 = np.bincount(key3, minlength=nco * WPC * 3).reshape(nco, WPC, 3)
    lo_only, flex, hi_only = cnt3[..., 0], cnt3[..., 1], cnt3[..., 2]
    total = lo_only + flex + hi_only
    Lmin = (-(-lo_only // P)).max(axis=0)
    Hmin = (-(-hi_only // P)).max(axis=0)
    Tmin = (-(-total // P)).max(axis=0)
    LplusH = np.maximum(np.maximum(Lmin + Hmin, Tmin), 1)
    Lb = Lmin + (LplusH - Lmin - Hmin) // 2
    budget = np.stack([Lb, LplusH - Lb], axis=1)   # [WPC, 2]
    T = int(budget.sum())

    tile_win = np.zeros(T, dtype=np.int64)
    wb_start = np.zeros((WPC, 2), dtype=np.int64)
    runs = []
    t = 0
    for g in range(cfg.NGROUPS):
        w0, w1 = g * cfg.GROUP_W, min((g + 1) * cfg.GROUP_W, WPC)
        for b in (0, 1):
            run_t0, run_n = t, 0
            for w in range(w0, w1):
                nb = int(budget[w, b])
                wb_start[w, b] = t
                tile_win[t:t + nb] = w
                t += nb
                run_n += nb
            # split evenly into runs of <= RUN_CAP tiles
            nrun = -(-run_n // cfg.RUN_CAP)
            for r in range(nrun):
                take = run_n // nrun + (1 if r < run_n % nrun else 0)
                runs.append((run_t0, take, b))
                run_t0 += take
    assert t == T

    tile_bank = (tile_win // cfg.GROUP_W) * 2 + (tile_win % cfg.GROUP_W) // cfg.BANK_W
    n_banks = int(tile_bank.max()) + 1
    bank_first = np.full(n_banks, T, dtype=np.int64)
    bank_last = np.full(n_banks, -1, dtype=np.int64)
    for i in range(T):
        bk = tile_bank[i]
        bank_first[bk] = min(bank_first[bk], i)
        bank_last[bk] = max(bank_last[bk], i)

    structure = dict(cfg=cfg, T=T, budget=budget, runs=runs,
                     tile_win=tile_win, tile_bank=tile_bank,
                     bank_first=bank_first, bank_last=bank_last)

    # per-core data: edges of (c, w) in order [lo_only..., flex..., hi_only...],
    # each col-sorted; the first P*L_w - lo_only flex edges go to the lo bucket.
    order = np.lexsort((col, cat, win, core))
    col_s = col[order]
    core_s, win_s = core[order], win[order]
    slot_s = slot[order]
    norm_s = norm[order]
    cw_count = np.zeros(nco * WPC, dtype=np.int64)
    np.add.at(cw_count, core_s * WPC + win_s, 1)
    cw_start = np.concatenate([[0], np.cumsum(cw_count)[:-1]]).reshape(nco, WPC)
    cw_count = cw_count.reshape(nco, WPC)

    per_core = []
    for c in range(nco):
        gidx = np.zeros(T * P, dtype=np.int16)
        sl = np.zeros((P, T), dtype=np.float32)
        nm = np.zeros((P, T), dtype=np.float32)
        for w in range(WPC):
            e0 = int(cw_start[c, w])
            ne = int(cw_count[c, w])
            nlo_only = int(lo_only[c, w])
            nflex = int(flex[c, w])
            Lw, Hw = int(budget[w, 0]), int(budget[w, 1])
            k = min(max(P * Lw - nlo_only, 0), nflex)
            n_lo = nlo_only + k
            n_hi = ne - n_lo
            assert n_lo <= P * Lw and n_hi <= P * Hw, (c, w, n_lo, n_hi, Lw, Hw)
            for b, (s0, nb_e, nb_t) in enumerate(
                    [(e0, n_lo, Lw), (e0 + n_lo, n_hi, Hw)]):
                if nb_t == 0:
                    continue
                t0 = int(wb_start[w, b])
                base = 0 if b == 0 else cfg.HI_BASE
                gp = t0 * P + np.arange(nb_e)
                gidx[gp] = (col_s[s0:s0 + nb_e] - base).astype(np.int16)
                pp, tt = gp % P, gp // P
                sl[pp, tt] = slot_s[s0:s0 + nb_e].astype(np.float32)
                nm[pp, tt] = norm_s[s0:s0 + nb_e]
        wrapped = gidx.reshape(-1, 16).T
        gidx128 = np.tile(wrapped, (8, 1))
        per_core.append(dict(gidx=gidx128, sl=sl, nm=nm))

    return structure, per_core


def _build_kernel(nc, tc, structure, aps):
    from concourse import mybir

    cfg = structure["cfg"]
    T = structure["T"]
    runs = structure["runs"]
    tile_win = structure["tile_win"]
    tile_bank = structure["tile_bank"]
    bank_first = structure["bank_first"]
    bank_last = structure["bank_last"]
    S, NPC, BANK_W, GROUP_W = cfg.S, cfg.NPC, cfg.BANK_W, cfg.GROUP_W
    WPS_BANK = BANK_W * S
    f32 = mybir.dt.float32

    x_ap = aps["x"]
    xdt = x_ap.dtype                     # float32r on HW, float32 in sim
    wdt = aps["wt"].dtype
    x_lo = x_ap[0:min(cfg.LO_LIM, cfg.N), :]
    x_hi = x_ap[cfg.HI_BASE:cfg.N, :]

    from contextlib import ExitStack
    ctx = ExitStack()
    const = ctx.enter_context(tc.tile_pool(name="const", bufs=1))
    gpool = ctx.enter_context(tc.tile_pool(name="gather", bufs=6))
    ohpool = ctx.enter_context(tc.tile_pool(name="onehot", bufs=6))
    pspool = ctx.enter_context(tc.tile_pool(name="psagg", bufs=6, space="PSUM"))
    psf = ctx.enter_context(tc.tile_pool(name="psfin", bufs=2, space="PSUM"))
    opool = ctx.enter_context(tc.tile_pool(name="ostage", bufs=3))

    # group tile ranges (tiles are laid out group-major)
    g_first = []
    g_count = []
    for g in range(cfg.NGROUPS):
        w0g, w1g = g * GROUP_W, min((g + 1) * GROUP_W, cfg.WPC)
        idxs = np.nonzero((tile_win >= w0g) & (tile_win < w1g))[0]
        g_first.append(int(idxs[0]))
        g_count.append(int(idxs.size))
    # per-group chunks of the gather indices / slot / norm streams so the
    # first group's gathers only wait on its own chunk
    gidx_g, sl_g, nm_g = [], [], []
    for g in range(cfg.NGROUPS):
        f0, nt_g = g_first[g], g_count[g]
        gi = const.tile([P, nt_g * 8], mybir.dt.int16, name=f"gidx{g}")
        nc.sync.dma_start(gi[:], aps["gidx"][:, f0 * 8:(f0 + nt_g) * 8])
        slg = const.tile([P, nt_g], f32, name=f"sl{g}")
        nc.sync.dma_start(slg[:], aps["sl"][:, f0:f0 + nt_g])
        nmg = const.tile([P, nt_g], f32, name=f"nm{g}")
        nc.sync.dma_start(nmg[:], aps["nm"][:, f0:f0 + nt_g])
        gidx_g.append(gi)
        sl_g.append(slg)
        nm_g.append(nmg)
    wt_sb = const.tile([P, C], wdt)
    nc.sync.dma_start(wt_sb[:], aps["wt"])
    bb_sb = const.tile([P, C], f32)
    nc.sync.dma_start(bb_sb[:], aps["bb"])
    iota_sb = const.tile([P, cfg.RUN_CAP, S], f32)
    nc.sync.dma_start(iota_sb[:], aps["iota"])
    aggT = const.tile([P, NPC], wdt)

    run_by_t0 = {r[0]: r for r in runs}
    # spread gather descriptor generation across the 4 SWDGE queues
    # (4 Q7 core-pairs generate in parallel)
    nq = getattr(nc, "num_swdge_queues", 1)
    run_qnum = {r[0]: i % nq for i, r in enumerate(runs)}

    for g in range(cfg.NGROUPS):
        w0 = g * GROUP_W
        w1 = min((g + 1) * GROUP_W, cfg.WPC)
        pbank = [None, None]

        def bank_tile(h, g=g):
            if pbank[h] is None:
                pbank[h] = pspool.tile([P, WPS_BANK], f32, tag="psbank",
                                       name=f"psbank_g{g}_h{h}")
            return pbank[h]

        tiles_of_group = [i for i in range(T) if w0 <= tile_win[i] < w1]
        cur = 0
        while cur < len(tiles_of_group):
            t0 = tiles_of_group[cur]
            run = run_by_t0.get(t0)
            assert run is not None, (g, t0)
            _, ntiles, b = run
            gbuf = gpool.tile([P, cfg.RUN_CAP, C], xdt, tag="gbuf")
            src = x_lo if b == 0 else x_hi
            tl = t0 - g_first[g]
            # stripe the run's gather across all 4 SWDGE queues so the
            # four Q7 core-pairs generate descriptors concurrently
            nq = max(1, getattr(nc, "num_swdge_queues", 1))
            nsub = min(nq, ntiles)
            o = 0
            for q in range(nsub):
                k = ntiles // nsub + (1 if q < ntiles % nsub else 0)
                nc.gpsimd.dma_gather(
                    gbuf[:, o:o + k, :], src,
                    gidx_g[g][:, (tl + o) * 8:(tl + o + k) * 8],
                    num_idxs=k * P, num_idxs_reg=k * P,
                    elem_size=C, elem_step=C, single_packet=False,
                    queue_num=q)
                o += k
            # one-hot for the whole run in two DVE ops:
            # oh[p, i, s] = (iota[s] == slot[p, t0+i]) * norm[p, t0+i]
            oh_run = ohpool.tile([P, cfg.RUN_CAP, S], xdt, tag="oh")
            slot_b = sl_g[g][:, tl:tl + ntiles].to_broadcast([P, ntiles, S])
            norm_b = nm_g[g][:, tl:tl + ntiles].to_broadcast([P, ntiles, S])
            nc.vector.tensor_tensor(
                oh_run[:, :ntiles, :], iota_sb[:, :ntiles, :], slot_b,
                op=mybir.AluOpType.is_equal)
            nc.vector.tensor_tensor(
                oh_run[:, :ntiles, :], oh_run[:, :ntiles, :], norm_b,
                op=mybir.AluOpType.mult)
            for i in range(ntiles):
                t = t0 + i
                w = int(tile_win[t])
                wl = w - w0
                pg = bank_tile(wl // BANK_W)
                off = (wl % BANK_W) * S
                bk = int(tile_bank[t])
                nc.tensor.matmul(
                    pg[:, off:off + S], lhsT=gbuf[:, i, :], rhs=oh_run[:, i, :],
                    start=(t == bank_first[bk]), stop=(t == bank_last[bk]),
                    skip_group_check=True)
            cur += ntiles

        for h in (0, 1):
            if pbank[h] is None:
                continue
            base = g * 2 * WPS_BANK + h * WPS_BANK
            nvalid = min(WPS_BANK, NPC - base)
            if nvalid <= 0:
                continue
            nc.scalar.copy(aggT[:, base:base + nvalid],
                           pbank[h][:, :nvalid])

    n_nt = -(-NPC // P)
    for nt in range(n_nt):
        n0 = nt * P
        m = min(P, NPC - n0)
        pf = psf.tile([P, C], f32, tag="pf")
        nc.tensor.matmul(pf[:m, :], lhsT=aggT[:, n0:n0 + m], rhs=wt_sb[:],
                         start=True, stop=True)
        ob = opool.tile([P, C], f32, tag="ob")
        nc.vector.tensor_add(ob[:m, :], pf[:m, :], bb_sb[:m, :])
        nc.sync.dma_start(aps["out"][n0:n0 + m, :], ob[:m, :])

    ctx.close()


_CACHE = {}


def _prepare(edge_index):
    key = hashlib.sha1(np.ascontiguousarray(edge_index).tobytes()).hexdigest()
    if key in _CACHE:
        return _CACHE[key]

    import concourse.bacc as bacc
    import concourse.tile as tile
    from concourse import mybir

    cfg = _Cfg(N_NODES, n_cores=N_CORES)
    structure, per_core = _build_plan(cfg, edge_index)
    T = structure["T"]

    nc = bacc.Bacc("TRN2", target_bir_lowering=False, debug=False,
                   num_devices=N_CORES, num_swdge_queues=4)
    f32 = mybir.dt.float32
    aps = {
        "x": nc.dram_tensor("x", [cfg.N, C], mybir.dt.float32r, kind="ExternalInput").ap(),
        "gidx": nc.dram_tensor("gidx", [P, T * 8], mybir.dt.int16,
                               kind="ExternalInput").ap(),
        "sl": nc.dram_tensor("sl", [P, T], f32, kind="ExternalInput").ap(),
        "nm": nc.dram_tensor("nm", [P, T], f32, kind="ExternalInput").ap(),
        "wt": nc.dram_tensor("wt", [P, C], mybir.dt.float32r, kind="ExternalInput").ap(),
        "bb": nc.dram_tensor("bb", [P, C], f32, kind="ExternalInput").ap(),
        "iota": nc.dram_tensor("iota", [P, cfg.RUN_CAP * cfg.S], f32,
                               kind="ExternalInput").ap(),
        "out": nc.dram_tensor("out", [cfg.NPC, C], f32,
                              kind="ExternalOutput").ap(),
    }
    with tile.TileContext(nc) as tc:
        _build_kernel(nc, tc, structure, aps)
    nc.compile()

    _CACHE[key] = (cfg, structure, per_core, nc)
    return _CACHE[key]


def _make_in_maps(cfg, structure, per_core, x, W, b):
    x32 = np.ascontiguousarray(x, dtype=np.float32)
    wt = np.ascontiguousarray(np.asarray(W, dtype=np.float32).T)
    bb = np.tile(np.asarray(b, dtype=np.float32), (P, 1))
    iota = np.tile(np.arange(cfg.S, dtype=np.float32), (P, cfg.RUN_CAP))
    in_maps = []
    for c in range(cfg.n_cores):
        pc = per_core[c]
        in_maps.append(dict(x=x32, gidx=pc["gidx"], sl=pc["sl"],
                            nm=pc["nm"], wt=wt, bb=bb, iota=iota))
    return in_maps


def kernel(x, edge_index, W, b):
    from concourse import bass_utils

    cfg, structure, per_core, nc = _prepare(np.asarray(edge_index))
    in_maps = _make_in_maps(cfg, structure, per_core, x, W, b)
    res = bass_utils.run_bass_kernel_spmd(nc, in_maps,
                                          core_ids=list(range(cfg.n_cores)))
    out = np.concatenate([res.results[c]["out"] for c in range(cfg.n_cores)],
                         axis=0)
    return out.astype(np.float32)


# revision 21
# speedup vs baseline: 1.2032x; 1.2032x over previous
"""GCNConv (N=50000, E=600000, C=128) on 8 TRN2 NeuronCores via Bass/Tile.

out = scatter_add(norm[e] * x[col[e]] -> row[e]) @ W.T + b,
norm[e] = deg^-1/2[row[e]] * deg^-1/2[col[e]]  (deg over row indices).

Strategy: shard by destination-node range (6250 nodes/core). Host sorts
edges by (core, dest-window, col-bucket) and pads to 128-edge tiles; the
per-(window,bucket) tile budgets are maxed over cores so a single SPMD
program (uniform instruction stream, per-core data) serves all 8 cores.

Per core: dma_gather fetches x rows (fp32, 512B descriptors); the DVE
builds a norm-valued one-hot [128 edges x 64 slots] per tile via
(iota == slot) * norm; the PE accumulates aggT[ch, dest] = msgs^T @
one-hot into PSUM banks (start/stop per bank; per-element has_written
gives overwrite-then-accumulate); banks are copied into an SBUF
aggT [128, 6250]; final PE matmuls apply W^T, a DVE add applies the
(partition-broadcast) bias, and rows DMA out. Output slices concatenate
to the full [50000, 128] result.
"""
import sys
import hashlib

if "/opt/trn_rl_repo" not in sys.path:
    sys.path.insert(0, "/opt/trn_rl_repo")

import numpy as np

P = 128
C = 128
N_NODES = 50000
N_EDGES = 600000
N_CORES = 8


class _Cfg:
    def __init__(self, n_nodes, n_cores=8, S=64, bank_w=8, run_cap=16,
                 lo_lim=32768):
        assert n_nodes % n_cores == 0
        self.N = n_nodes
        self.n_cores = n_cores
        self.NPC = n_nodes // n_cores
        self.S = S
        self.BANK_W = bank_w
        self.GROUP_W = 2 * bank_w
        self.WPC = -(-self.NPC // S)
        self.NGROUPS = -(-self.WPC // self.GROUP_W)
        self.RUN_CAP = run_cap
        self.LO_LIM = min(lo_lim, n_nodes)
        self.HI_BASE = max(0, n_nodes - 32768)
        assert self.LO_LIM <= 32768
        assert self.N - self.HI_BASE <= 32768
        assert self.LO_LIM > self.HI_BASE or self.N <= self.LO_LIM


def _build_plan(cfg, edge_index):
    row = np.asarray(edge_index[0], dtype=np.int64)
    col = np.asarray(edge_index[1], dtype=np.int64)
    N, NPC, S, WPC = cfg.N, cfg.NPC, cfg.S, cfg.WPC
    nco = cfg.n_cores

    deg = np.bincount(row, minlength=N).astype(np.float64)
    dis = np.where(deg > 0, 1.0 / np.sqrt(np.maximum(deg, 1.0)), 0.0)
    norm = (dis[row] * dis[col]).astype(np.float32)

    core = row // NPC
    r_loc = row - core * NPC
    win = r_loc // S
    slot = r_loc % S
    # category: 0 = lo-only (< HI_BASE), 1 = flexible, 2 = hi-only (>= LO_LIM)
    cat = np.where(col < cfg.HI_BASE, 0, np.where(col < cfg.LO_LIM, 1, 2))

    key3 = (core * WPC + win) * 3 + cat
    cnt3 = np.bincount(key3, minlength=nco * WPC * 3).reshape(nco, WPC, 3)
    lo_only, flex, hi_only = cnt3[..., 0], cnt3[..., 1], cnt3[..., 2]
    total = lo_only + flex + hi_only
    Lmin = (-(-lo_only // P)).max(axis=0)
    Hmin = (-(-hi_only // P)).max(axis=0)
    Tmin = (-(-total // P)).max(axis=0)
    LplusH = np.maximum(np.maximum(Lmin + Hmin, Tmin), 1)
    Lb = Lmin + (LplusH - Lmin - Hmin) // 2
    budget = np.stack([Lb, LplusH - Lb], axis=1)   # [WPC, 2]
    T = int(budget.sum())

    tile_win = np.zeros(T, dtype=np.int64)
    wb_start = np.zeros((WPC, 2), dtype=np.int64)
    runs = []
    t = 0
    for g in range(cfg.NGROUPS):
        w0, w1 = g * cfg.GROUP_W, min((g + 1) * cfg.GROUP_W, WPC)
        for b in (0, 1):
            run_t0, run_n = t, 0
            for w in range(w0, w1):
                nb = int(budget[w, b])
                wb_start[w, b] = t
                tile_win[t:t + nb] = w
                t += nb
                run_n += nb
            # split evenly into runs of <= RUN_CAP tiles
            nrun = -(-run_n // cfg.RUN_CAP)
            for r in range(nrun):
                take = run_n // nrun + (1 if r < run_n % nrun else 0)
                runs.append((run_t0, take, b))
                run_t0 += take
    assert t == T

    tile_bank = (tile_win // cfg.GROUP_W) * 2 + (tile_win % cfg.GROUP_W) // cfg.BANK_W
    n_banks = int(tile_bank.max()) + 1
    bank_first = np.full(n_banks, T, dtype=np.int64)
    bank_last = np.full(n_banks, -1, dtype=np.int64)
    for i in range(T):
        bk = tile_bank[i]
        bank_first[bk] = min(bank_first[bk], i)
        bank_last[bk] = max(bank_last[bk], i)

    structure = dict(cfg=cfg, T=T, budget=budget, runs=runs,
                     tile_win=tile_win, tile_bank=tile_bank,
                     bank_first=bank_first, bank_last=bank_last)

    # per-core data: edges of (c, w) in order [lo_only..., flex..., hi_only...],
    # each col-sorted; the first P*L_w - lo_only flex edges go to the lo bucket.
    order = np.lexsort((col, cat, win, core))
    col_s = col[order]
    core_s, win_s = core[order], win[order]
    slot_s = slot[order]
    norm_s = norm[order]
    cw_count = np.zeros(nco * WPC, dtype=np.int64)
    np.add.at(cw_count, core_s * WPC + win_s, 1)
    cw_start = np.concatenate([[0], np.cumsum(cw_count)[:-1]]).reshape(nco, WPC)
    cw_count = cw_count.reshape(nco, WPC)

    per_core = []
    for c in range(nco):
        gidx = np.zeros(T * P, dtype=np.int16)
        sl = np.zeros((P, T), dtype=np.float32)
        nm = np.zeros((P, T), dtype=np.float32)
        for w in range(WPC):
            e0 = int(cw_start[c, w])
            ne = int(cw_count[c, w])
            nlo_only = int(lo_only[c, w])
            nflex = int(flex[c, w])
            Lw, Hw = int(budget[w, 0]), int(budget[w, 1])
            k = min(max(P * Lw - nlo_only, 0), nflex)
            n_lo = nlo_only + k
            n_hi = ne - n_lo
            assert n_lo <= P * Lw and n_hi <= P * Hw, (c, w, n_lo, n_hi, Lw, Hw)
            for b, (s0, nb_e, nb_t) in enumerate(
                    [(e0, n_lo, Lw), (e0 + n_lo, n_hi, Hw)]):
                if nb_t == 0:
                    continue
                t0 = int(wb_start[w, b])
                base = 0 if b == 0 else cfg.HI_BASE
                gp = t0 * P + np.arange(nb_e)
                gidx[gp] = (col_s[s0:s0 + nb_e] - base).astype(np.int16)
                pp, tt = gp % P, gp // P
                sl[pp, tt] = slot_s[s0:s0 + nb_e].astype(np.float32)
                nm[pp, tt] = norm_s[s0:s0 + nb_e]
        wrapped = gidx.reshape(-1, 16).T
        gidx128 = np.tile(wrapped, (8, 1))
        per_core.append(dict(gidx=gidx128, sl=sl, nm=nm))

    return structure, per_core


def _build_kernel(nc, tc, structure, aps):
    from concourse import mybir

    cfg = structure["cfg"]
    T = structure["T"]
    runs = structure["runs"]
    tile_win = structure["tile_win"]
    tile_bank = structure["tile_bank"]
    bank_first = structure["bank_first"]
    bank_last = structure["bank_last"]
    S, NPC, BANK_W, GROUP_W = cfg.S, cfg.NPC, cfg.BANK_W, cfg.GROUP_W
    WPS_BANK = BANK_W * S
    f32 = mybir.dt.float32

    x_ap = aps["x"]
    xdt = x_ap.dtype                     # float32r on HW, float32 in sim
    wdt = aps["wt"].dtype
    x_lo = x_ap[0:min(cfg.LO_LIM, cfg.N), :]
    x_hi = x_ap[cfg.HI_BASE:cfg.N, :]

    from contextlib import ExitStack
    ctx = ExitStack()
    const = ctx.enter_context(tc.tile_pool(name="const", bufs=1))
    gpool = ctx.enter_context(tc.tile_pool(name="gather", bufs=6))
    ohpool = ctx.enter_context(tc.tile_pool(name="onehot", bufs=6))
    pspool = ctx.enter_context(tc.tile_pool(name="psagg", bufs=6, space="PSUM"))
    psf = ctx.enter_context(tc.tile_pool(name="psfin", bufs=2, space="PSUM"))
    opool = ctx.enter_context(tc.tile_pool(name="ostage", bufs=3))

    # group tile ranges (tiles are laid out group-major)
    g_first = []
    g_count = []
    for g in range(cfg.NGROUPS):
        w0g, w1g = g * GROUP_W, min((g + 1) * GROUP_W, cfg.WPC)
        idxs = np.nonzero((tile_win >= w0g) & (tile_win < w1g))[0]
        g_first.append(int(idxs[0]))
        g_count.append(int(idxs.size))
    # per-group chunks of the gather indices / slot / norm streams so the
    # first group's gathers only wait on its own chunk
    gidx_g, sl_g, nm_g = [], [], []
    for g in range(cfg.NGROUPS):
        f0, nt_g = g_first[g], g_count[g]
        gi = const.tile([P, nt_g * 8], mybir.dt.int16, name=f"gidx{g}")
        nc.sync.dma_start(gi[:], aps["gidx"][:, f0 * 8:(f0 + nt_g) * 8])
        slg = const.tile([P, nt_g], f32, name=f"sl{g}")
        nc.sync.dma_start(slg[:], aps["sl"][:, f0:f0 + nt_g])
        nmg = const.tile([P, nt_g], f32, name=f"nm{g}")
        nc.sync.dma_start(nmg[:], aps["nm"][:, f0:f0 + nt_g])
        gidx_g.append(gi)
        sl_g.append(slg)
        nm_g.append(nmg)
    wt_sb = const.tile([P, C], wdt)
    nc.sync.dma_start(wt_sb[:], aps["wt"])
    bb_sb = const.tile([P, C], f32)
    nc.sync.dma_start(bb_sb[:], aps["bb"])
    iota_sb = const.tile([P, cfg.RUN_CAP, S], f32)
    nc.sync.dma_start(iota_sb[:], aps["iota"])
    aggT = const.tile([P, NPC], wdt)

    run_by_t0 = {r[0]: r for r in runs}
    # spread gather descriptor generation across the 4 SWDGE queues
    # (4 Q7 core-pairs generate in parallel)
    nq = getattr(nc, "num_swdge_queues", 1)
    run_qnum = {r[0]: i % nq for i, r in enumerate(runs)}

    for g in range(cfg.NGROUPS):
        w0 = g * GROUP_W
        w1 = min((g + 1) * GROUP_W, cfg.WPC)
        pbank = [None, None]

        def bank_tile(h, g=g):
            if pbank[h] is None:
                pbank[h] = pspool.tile([P, WPS_BANK], f32, tag="psbank",
                                       name=f"psbank_g{g}_h{h}")
            return pbank[h]

        tiles_of_group = [i for i in range(T) if w0 <= tile_win[i] < w1]
        cur = 0
        while cur < len(tiles_of_group):
            t0 = tiles_of_group[cur]
            run = run_by_t0.get(t0)
            assert run is not None, (g, t0)
            _, ntiles, b = run
            gbuf = gpool.tile([P, cfg.RUN_CAP, C], xdt, tag="gbuf")
            src = x_lo if b == 0 else x_hi
            tl = t0 - g_first[g]
            nc.gpsimd.dma_gather(
                gbuf[:, :ntiles, :], src,
                gidx_g[g][:, tl * 8:(tl + ntiles) * 8],
                num_idxs=ntiles * P, num_idxs_reg=ntiles * P,
                elem_size=C, elem_step=C, single_packet=False,
                queue_num=run_qnum.get(t0, 0))
            # one-hot for the whole run in two DVE ops:
            # oh[p, i, s] = (iota[s] == slot[p, t0+i]) * norm[p, t0+i]
            oh_run = ohpool.tile([P, cfg.RUN_CAP, S], xdt, tag="oh")
            slot_b = sl_g[g][:, tl:tl + ntiles].to_broadcast([P, ntiles, S])
            norm_b = nm_g[g][:, tl:tl + ntiles].to_broadcast([P, ntiles, S])
            nc.vector.tensor_tensor(
                oh_run[:, :ntiles, :], iota_sb[:, :ntiles, :], slot_b,
                op=mybir.AluOpType.is_equal)
            nc.vector.tensor_tensor(
                oh_run[:, :ntiles, :], oh_run[:, :ntiles, :], norm_b,
                op=mybir.AluOpType.mult)
            for i in range(ntiles):
                t = t0 + i
                w = int(tile_win[t])
                wl = w - w0
                pg = bank_tile(wl // BANK_W)
                off = (wl % BANK_W) * S
                bk = int(tile_bank[t])
                nc.tensor.matmul(
                    pg[:, off:off + S], lhsT=gbuf[:, i, :], rhs=oh_run[:, i, :],
                    start=(t == bank_first[bk]), stop=(t == bank_last[bk]),
                    skip_group_check=True)
            cur += ntiles

        for h in (0, 1):
            if pbank[h] is None:
                continue
            base = g * 2 * WPS_BANK + h * WPS_BANK
            nvalid = min(WPS_BANK, NPC - base)
            if nvalid <= 0:
                continue
            nc.scalar.copy(aggT[:, base:base + nvalid],
                           pbank[h][:, :nvalid])

    n_nt = -(-NPC // P)
    for nt in range(n_nt):
        n0 = nt * P
        m = min(P, NPC - n0)
        pf = psf.tile([P, C], f32, tag="pf")
        nc.tensor.matmul(pf[:m, :], lhsT=aggT[:, n0:n0 + m], rhs=wt_sb[:],
                         start=True, stop=True)
        ob = opool.tile([P, C], f32, tag="ob")
        nc.vector.tensor_add(ob[:m, :], pf[:m, :], bb_sb[:m, :])
        nc.sync.dma_start(aps["out"][n0:n0 + m, :], ob[:m, :])

    ctx.close()


_CACHE = {}


def _prepare(edge_index):
    key = hashlib.sha1(np.ascontiguousarray(edge_index).tobytes()).hexdigest()
    if key in _CACHE:
        return _CACHE[key]

    import concourse.bacc as bacc
    import concourse.tile as tile
    from concourse import mybir

    cfg = _Cfg(N_NODES, n_cores=N_CORES)
    structure, per_core = _build_plan(cfg, edge_index)
    T = structure["T"]

    nc = bacc.Bacc("TRN2", target_bir_lowering=False, debug=False,
                   num_devices=N_CORES, num_swdge_queues=4)
    f32 = mybir.dt.float32
    aps = {
        "x": nc.dram_tensor("x", [cfg.N, C], mybir.dt.float32r, kind="ExternalInput").ap(),
        "gidx": nc.dram_tensor("gidx", [P, T * 8], mybir.dt.int16,
                               kind="ExternalInput").ap(),
        "sl": nc.dram_tensor("sl", [P, T], f32, kind="ExternalInput").ap(),
        "nm": nc.dram_tensor("nm", [P, T], f32, kind="ExternalInput").ap(),
        "wt": nc.dram_tensor("wt", [P, C], mybir.dt.float32r, kind="ExternalInput").ap(),
        "bb": nc.dram_tensor("bb", [P, C], f32, kind="ExternalInput").ap(),
        "iota": nc.dram_tensor("iota", [P, cfg.RUN_CAP * cfg.S], f32,
                               kind="ExternalInput").ap(),
        "out": nc.dram_tensor("out", [cfg.NPC, C], f32,
                              kind="ExternalOutput").ap(),
    }
    with tile.TileContext(nc) as tc:
        _build_kernel(nc, tc, structure, aps)
    nc.compile()

    _CACHE[key] = (cfg, structure, per_core, nc)
    return _CACHE[key]


def _make_in_maps(cfg, structure, per_core, x, W, b):
    x32 = np.ascontiguousarray(x, dtype=np.float32)
    wt = np.ascontiguousarray(np.asarray(W, dtype=np.float32).T)
    bb = np.tile(np.asarray(b, dtype=np.float32), (P, 1))
    iota = np.tile(np.arange(cfg.S, dtype=np.float32), (P, cfg.RUN_CAP))
    in_maps = []
    for c in range(cfg.n_cores):
        pc = per_core[c]
        in_maps.append(dict(x=x32, gidx=pc["gidx"], sl=pc["sl"],
                            nm=pc["nm"], wt=wt, bb=bb, iota=iota))
    return in_maps


def kernel(x, edge_index, W, b):
    from concourse import bass_utils

    cfg, structure, per_core, nc = _prepare(np.asarray(edge_index))
    in_maps = _make_in_maps(cfg, structure, per_core, x, W, b)
    res = bass_utils.run_bass_kernel_spmd(nc, in_maps,
                                          core_ids=list(range(cfg.n_cores)))
    out = np.concatenate([res.results[c]["out"] for c in range(cfg.n_cores)],
                         axis=0)
    return out.astype(np.float32)
